# revision 27
# baseline (speedup 1.0000x reference)
"""Qwen3-style GQA attention (B=1, S=2048, DM=2048, H=16, KV=4, D=128) on 8 TRN2 cores.

Sharding: tensor-parallel over heads. Core c computes Q heads {2c, 2c+1} and
KV head c//2 end-to-end, then a partial output hs_part = gated_local @ Wo_rows.
Host sums the 8 partials.

Precision scheme (v2, single-term): all projections and scores run in plain
float32r (11-bit-mantissa operands, exact products, fp32 PSUM accumulation) at
full PE rate; Q/K-path operands are pre-rounded on the host so engine-side
rounding mode cannot bias them. The resulting logit noise is ~0.05 on logits
whose top-2 gap is ~20, giving worst-case output error ~1e-2 relative (numpy
precision sim) against the 2e-2 gate. The attention tail (exp weights,
diag(1/Z), P^T, V) runs in fp16 (4x tighter than bf16, same 1-cycle/row PE
rate; pu <= ~e^0.5 so fp16 range is safe). 1/Z rounding is row-uniform
(absorbed by softmax temperature). RMSNorm: sum-of-squares + Q-side 1/rms
broadcast in f32r (Q-side scale errors are row-uniform -> harmless); K-side
broadcast stays exact fp32 since per-column K scale errors hit tie logits
multiplied by |logit| (~400).

Structure: one fused projection pass streams hsT chunks once and computes all
six outputs, with RMSNorm + RoPE fused per 512-slice (deferred one slice so
norm chains overlap the next slice's matmuls). RoPE's rotate-half is an exact
partition-offset SBUF->SBUF DMA (sign folded into a host-prepared signed sin
table). Attention runs both heads interleaved per q-chunk; a bf16 score
pre-pass provides the softmax max so the f32r scores go matmul->exp with no
reduce in between. P^T for AV comes from an fp16 matmul against diag(1/Z),
fusing normalization into the transpose; the AV accumulate is software-
pipelined one k-block behind the P^T matmuls so the PE never waits on the
PSUM->SBUF copy. Wo partial matmuls for q-chunk qc are interleaved into
q-chunk qc+1's softmax rounds to fill exp/reduce latency.
"""

import numpy as np

S = 2048
DM = 2048
D = 128
HPC = 2           # q heads per core
NCORES = 8
SCALING = float(D) ** 0.5
EPS = 1e-6
P = 128
KCH = DM // P     # 16 contraction chunks for projections
NQB = S // P      # 16 q blocks
NSC = S // 512    # 4 seq chunks of 512

_cache = {}


def _round_fp32r(x):
    x = np.ascontiguousarray(x, dtype=np.float32)
    b = x.view(np.uint32).astype(np.uint64)
    lsb = (b >> 12) & 1
    r = (b + 0x7FF + lsb) & 0xFFFFF000
    return r.astype(np.uint32).view(np.float32)


def _build_nc():
    import concourse.tile as tile
    from concourse import bacc, mybir

    F32 = mybir.dt.float32
    F32R = mybir.dt.float32r
    BF16 = mybir.dt.bfloat16
    FP16 = mybir.dt.float16
    AF = mybir.ActivationFunctionType
    from concourse.alu_op_type import AluOpType as ALU
    AX = mybir.AxisListType.X

    nc = bacc.Bacc(None, target_bir_lowering=False, debug=False)

    with nc.allow_low_precision(reason="f32r/fp16 operands are a deliberate "
                                "precision/speed tradeoff"), \
         tile.TileContext(nc) as tc:
        with tc.tile_pool(name="dram", bufs=1, space="DRAM") as dram:
            hsT = dram.tile([DM, S], F32R, kind="ExternalInput", name="hsT", uniquify=False)
            wq = dram.tile([DM, HPC * P], F32R, kind="ExternalInput", name="wq", uniquify=False)
            wk = dram.tile([DM, P], F32R, kind="ExternalInput", name="wk", uniquify=False)
            wg = dram.tile([DM, HPC * P], F32R, kind="ExternalInput", name="wg", uniquify=False)
            wv = dram.tile([DM, P], F32R, kind="ExternalInput", name="wv", uniquify=False)
            wo = dram.tile([HPC * P, DM], FP16, kind="ExternalInput", name="wo", uniquify=False)
            cosT = dram.tile([P, S], F32, kind="ExternalInput", name="cosT", uniquify=False)
            sinTs = dram.tile([P, S], F32, kind="ExternalInput", name="sinTs", uniquify=False)
            qw = dram.tile([P, 1], F32, kind="ExternalInput", name="qw", uniquify=False)
            kw = dram.tile([P, 1], F32, kind="ExternalInput", name="kw", uniquify=False)
            ident = dram.tile([P, P], F32R, kind="ExternalInput", name="ident", uniquify=False)
            identf = dram.tile([P, P], F32, kind="ExternalInput", name="identf", uniquify=False)
            onec = dram.tile([P, 1], F32R, kind="ExternalInput", name="onec", uniquify=False)
            oner = dram.tile([1, P], F32, kind="ExternalInput", name="oner", uniquify=False)
            onerr = dram.tile([1, P], F32R, kind="ExternalInput", name="onerr", uniquify=False)
            triu = dram.tile([P, P], F32, kind="ExternalInput", name="triu", uniquify=False)
            mask5 = dram.tile([P, 5 * 512], F32, kind="ExternalInput", name="mask5", uniquify=False)
            out = dram.tile([S, DM], FP16, kind="ExternalOutput", name="out", uniquify=False)

        # persistent SBUF (whole kernel)
        with tc.tile_pool(name="persist", bufs=1) as pers:
            qw_sb = pers.tile([P, 1], F32)
            kw_sb = pers.tile([P, 1], F32)
            ident_sb = pers.tile([P, P], F32R)
            identf_sb = pers.tile([P, P], F32)
            onec_sb = pers.tile([P, 1], F32R)
            oner_sb = pers.tile([1, P], F32)
            onerr_sb = pers.tile([1, P], F32R)
            triu_sb = pers.tile([P, P], F32)
            mask5_sb = pers.tile([P, 5 * 512], F32)
            eps_sb = pers.tile([1, 1], F32)
            k_hi = pers.tile([P, S], F32R)
            k_hb = pers.tile([P, S], BF16)
            q_hi = pers.tile([P, HPC, S], F32R)
            q_hb = pers.tile([P, HPC, S], BF16)
            sig_r = pers.tile([P, HPC, S], BF16)    # sigmoid(gate)
            vts = pers.tile([P, S], F32R)           # V untransposed [d, spos]
            v_r = pers.tile([P, NQB, P], FP16)      # V^T in s-major blocks
            gated_r = pers.tile([P, HPC, S], F32R)

            nc.sync.dma_start(qw_sb[:], qw[:])
            nc.sync.dma_start(kw_sb[:], kw[:])
            nc.sync.dma_start(ident_sb[:], ident[:])
            nc.sync.dma_start(identf_sb[:], identf[:])
            nc.sync.dma_start(onec_sb[:], onec[:])
            nc.sync.dma_start(oner_sb[:], oner[:])
            nc.sync.dma_start(onerr_sb[:], onerr[:])
            nc.sync.dma_start(triu_sb[:], triu[:])
            nc.sync.dma_start(mask5_sb[:], mask5[:])
            nc.gpsimd.memset(eps_sb[:], EPS)

            hsTr = hsT.rearrange("(kc p) s -> p kc s", p=P)

            # ====== P1 (fused): all projections + norm + rope ======
            with (
                tc.tile_pool(name="wts", bufs=1) as wpool,
                tc.tile_pool(name="cs", bufs=2) as cspool,
                tc.tile_pool(name="hs1", bufs=6) as hspool,
                tc.tile_pool(name="nsc", bufs=4) as nsc,
                tc.tile_pool(name="nxr", bufs=6) as nxr,
                tc.tile_pool(name="rr1", bufs=2) as rr1,
                tc.tile_pool(name="vts", bufs=1) as vtp,
                tc.tile_pool(name="pqk", bufs=1, space="PSUM") as pqk,
                tc.tile_pool(name="pnm", bufs=1, space="PSUM") as pnm,
            ):
                wq_sb = wpool.tile([P, KCH, HPC * P], F32R)
                wk_sb = wpool.tile([P, KCH, P], F32R)
                wg_sb = wpool.tile([P, KCH, HPC * P], F32R)
                wv_sb = wpool.tile([P, KCH, P], F32R)
                # kc=0 slices first so the first matmuls start immediately,
                # then the rest in quarter-tensor chunks
                wsrcs = ((wq_sb, wq), (wk_sb, wk), (wg_sb, wg), (wv_sb, wv))
                for (dst, src) in wsrcs:
                    nc.gpsimd.dma_start(
                        dst[:, 0:1, :],
                        src.rearrange("(kc p) m -> p kc m", p=P)[:, 0:1, :])
                for k4 in range(4):
                    ksl = slice(max(k4 * 4, 1), k4 * 4 + 4)
                    for (dst, src) in wsrcs:
                        nc.gpsimd.dma_start(
                            dst[:, ksl, :],
                            src.rearrange("(kc p) m -> p kc m", p=P)[:, ksl, :])

                pend = []

                def norm_a(job):
                    # stage A: sumsq matmul -> sqrt -> reciprocal. Emitted
                    # between projection chunk groups so the PE-queue matmul
                    # never waits on the serial vector/scalar chain (a >3.4us
                    # PE gap here re-throttles the HAM clock gate to 1.2GHz).
                    (xr, wvec, xhi, xhb, cos_t, sin_t, exact_bcast) = job
                    sqf = nsc.tile([P, 512], F32R, tag="scr", name="sqf")
                    nc.vector.tensor_mul(sqf[:], xr[:], xr[:])
                    # f32r sumsq (rel err ~1e-5 on the scale -> harmless)
                    ps1 = pnm.tile([1, 512], F32, tag="ps1")
                    nc.tensor.matmul(ps1[:], lhsT=onec_sb[:], rhs=sqf[:],
                                     start=True, stop=True)
                    sqv = rr1.tile([1, 512], F32, tag="sqv")
                    nc.scalar.activation(sqv[:], ps1[:], AF.Sqrt,
                                         scale=1.0 / D, bias=eps_sb[:])
                    if exact_bcast:
                        rr = rr1.tile([1, 512], F32, tag="rrk")
                    else:
                        rr = rr1.tile([1, 512], F32R, tag="rrq")
                    nc.vector.reciprocal(rr[:], sqv[:])
                    job.append(rr)

                def norm_b(job):
                    # stage B: 1/rms broadcast + scale + rope; emitted a few
                    # chunk groups after stage A so rr is long since ready
                    (xr, wvec, xhi, xhb, cos_t, sin_t, exact_bcast, rr) = job
                    psb = pnm.tile([P, 512], F32, tag="psb")
                    if exact_bcast:
                        # K-side: per-column scale errors hit tie logits x|S|;
                        # keep the broadcast exact (fp32-mode matmul)
                        nc.tensor.matmul(psb[:], lhsT=oner_sb[:], rhs=rr[:],
                                         start=True, stop=True)
                    else:
                        # Q-side: scale errors are row-uniform in softmax ->
                        # harmless; run at f32r rate
                        nc.tensor.matmul(psb[:], lhsT=onerr_sb[:], rhs=rr[:],
                                         start=True, stop=True)
                    xn = nsc.tile([P, 512], F32, tag="scr", name="xn")
                    nc.vector.scalar_tensor_tensor(
                        xn[:], xr[:], wvec[:], psb[:], op0=ALU.mult, op1=ALU.mult)
                    # rotate-half via partition-offset SBUF DMA (exact)
                    rot = nsc.tile([P, 512], F32, tag="scr", name="rot")
                    nc.scalar.dma_start(rot[0:64, :], xn[64:128, :])
                    nc.scalar.dma_start(rot[64:128, :], xn[0:64, :])
                    t2 = nsc.tile([P, 512], F32, tag="scr", name="t2")
                    nc.vector.tensor_mul(t2[:], rot[:], sin_t[:])
                    t1 = nsc.tile([P, 512], F32, tag="scr", name="t1")
                    nc.gpsimd.tensor_mul(t1[:], xn[:], cos_t[:])
                    xf = nsc.tile([P, 512], F32, tag="scr", name="xf")
                    nc.vector.tensor_add(xf[:], t1[:], t2[:])
                    nc.any.tensor_copy(xhi, xf[:])
                    nc.vector.tensor_copy(xhb, xf[:])

                for sq in range(NSC):
                    s0 = sq * 512
                    sl = slice(s0, s0 + 512)
                    cos_t = cspool.tile([P, 512], F32, tag="cos")
                    sin_t = cspool.tile([P, 512], F32, tag="sin")
                    nc.scalar.dma_start(cos_t[:], cosT[:, sl])
                    nc.scalar.dma_start(sin_t[:], sinTs[:, sl])
                    ps_q0 = pqk.tile([P, 512], F32, tag="psq0")
                    ps_q1 = pqk.tile([P, 512], F32, tag="psq1")
                    ps_k = pqk.tile([P, 512], F32, tag="psk")
                    ps_g0 = pqk.tile([P, 512], F32, tag="psg0")
                    ps_g1 = pqk.tile([P, 512], F32, tag="psg1")
                    ps_v = pqk.tile([P, 512], F32, tag="psv")
                    for kc in range(KCH):
                        hh = hspool.tile([P, 512], F32R, tag="hh")
                        # alternate queues so the 256KB chunks stream on two
                        # DMA engines
                        if kc % 2 == 0:
                            nc.sync.dma_start(hh[:], hsTr[:, kc, sl])
                        else:
                            nc.scalar.dma_start(hh[:], hsTr[:, kc, sl])
                        # fp16 copy feeds gate/V: 16-bit stationary weights get
                        # an overlapped LDWEIGHTS instead of f32r's serial
                        # in-instruction self-load
                        hf = hspool.tile([P, 512], FP16, tag="hf")
                        nc.vector.tensor_copy(hf[:], hh[:].bitcast(F32))
                        st = kc == 0
                        sp = kc == KCH - 1
                        nc.tensor.matmul(ps_q0[:], lhsT=wq_sb[:, kc, 0:P],
                                         rhs=hh[:], start=st, stop=sp)
                        nc.tensor.matmul(ps_q1[:], lhsT=wq_sb[:, kc, P:2 * P],
                                         rhs=hh[:], start=st, stop=sp)
                        nc.tensor.matmul(ps_k[:], lhsT=wk_sb[:, kc, :],
                                         rhs=hh[:], start=st, stop=sp)
                        nc.tensor.matmul(ps_g0[:], lhsT=wg_sb[:, kc, 0:P],
                                         rhs=hf[:], start=st, stop=sp)
                        nc.tensor.matmul(ps_g1[:], lhsT=wg_sb[:, kc, P:2 * P],
                                         rhs=hf[:], start=st, stop=sp)
                        nc.tensor.matmul(ps_v[:], lhsT=wv_sb[:, kc, :],
                                         rhs=hf[:], start=st, stop=sp)
                        # previous slice's norm/rope chains, staged between
                        # chunk groups so their matmuls never stall the PE
                        if pend:
                            if kc == 2:
                                norm_a(pend[0])
                            elif kc == 4:
                                norm_a(pend[1])
                            elif kc == 6:
                                norm_a(pend[2])
                            elif kc == 9:
                                norm_b(pend[0])
                            elif kc == 11:
                                norm_b(pend[1])
                            elif kc == 13:
                                norm_b(pend[2])
                    # gate: sigmoid straight off PSUM
                    nc.scalar.activation(sig_r[:, 0, sl], ps_g0[:], AF.Sigmoid)
                    nc.scalar.activation(sig_r[:, 1, sl], ps_g1[:], AF.Sigmoid)
                    # V: copy out untransposed; transposed in P2 preamble
                    nc.any.tensor_copy(vts[:, sl], ps_v[:])
                    # Q/K: copy raw projections out now (frees PSUM); the
                    # norm/rope chain is deferred one sq iteration so the next
                    # projection block hides its PE matmuls' input latency
                    newjobs = []
                    for (psd, wvec, xhi, xhb, exb) in (
                        (ps_q0, qw_sb, q_hi[:, 0, sl], q_hb[:, 0, sl], False),
                        (ps_q1, qw_sb, q_hi[:, 1, sl], q_hb[:, 1, sl], False),
                        (ps_k, kw_sb, k_hi[:, sl], k_hb[:, sl], True),
                    ):
                        xr = nxr.tile([P, 512], F32, tag="xr")
                        nc.any.tensor_copy(xr[:], psd[:])
                        newjobs.append([xr, wvec, xhi, xhb, cos_t, sin_t, exb])
                    del pend[:]
                    pend.extend(newjobs)
                for job in pend:
                    norm_a(job)
                for job in pend:
                    norm_b(job)
                del pend[:]

            # ====== P2: attention, heads interleaved, Wo pipelined in ======
            with (
                tc.tile_pool(name="shp", bufs=2, space="PSUM") as shp,
                tc.tile_pool(name="scp", bufs=3, space="PSUM") as scp,
                tc.tile_pool(name="pwo", bufs=2, space="PSUM") as pwo,
                tc.tile_pool(name="otp", bufs=1, space="PSUM") as otp,
                tc.tile_pool(name="pu", bufs=8) as pupool,
                tc.tile_pool(name="dd", bufs=10) as ddpool,
                tc.tile_pool(name="sm", bufs=16) as smpool,
                tc.tile_pool(name="pts", bufs=3) as ptspool,
                tc.tile_pool(name="wop", bufs=1) as wopool,
                tc.tile_pool(name="co", bufs=4) as copool,
            ):
                wo_sb = wopool.tile([P, HPC, DM], FP16)
                nc.sync.dma_start(wo_sb[:], wo.rearrange("(h p) m -> p h m", p=P))

                # V^T blocks off the persistent untransposed copy
                for b in range(NQB):
                    pst = shp.tile([P, P], F32R, tag="shared", name="pst")
                    nc.tensor.transpose(pst[:], vts[:, b * P:(b + 1) * P],
                                        ident_sb[:])
                    nc.any.tensor_copy(v_r[:, b, :], pst[:])

                wo_jobs = []

                def emit_wo(njobs):
                    for _ in range(min(njobs, len(wo_jobs))):
                        sb, dc = wo_jobs.pop(0)
                        pso = pwo.tile([P, 512], F32, name="pso")
                        for h in range(HPC):
                            nc.tensor.matmul(
                                pso[:],
                                lhsT=gated_r[:, h, sb * P:(sb + 1) * P],
                                rhs=wo_sb[:, h, dc * 512:(dc + 1) * 512],
                                start=(h == 0), stop=(h == HPC - 1))
                        cpo = copool.tile([P, 512], FP16)
                        nc.any.tensor_copy(cpo[:], pso[:])
                        nc.scalar.dma_start(
                            out[sb * P:(sb + 1) * P, dc * 512:(dc + 1) * 512],
                            cpo[:])

                for qc in (3, 2, 1, 0):
                    nfull = qc
                    kmax = 4 * qc + 3
                    pu_l = {}
                    d_l = {}
                    # interleave the two heads' per-qb softmax chains so one
                    # head's matmuls fill the other's reduce/exp latency
                    for qbi in range(4):
                        for h in range(HPC):
                            qb = 4 * qc + qbi
                            r = qb % 4
                            qsl = slice(qb * P, (qb + 1) * P)
                            # --- bf16 max pre-pass: approximate row max ---
                            mparts = smpool.tile([P, 8], F32, tag="mp")
                            for kc in range(nfull + 1):
                                w = 512 if kc < nfull else (r + 1) * P
                                ksl = slice(kc * 512, kc * 512 + w)
                                mx = shp.tile([P, 512], F32, tag="shared", name="mx")
                                nc.tensor.matmul(mx[:, :w], lhsT=q_hb[:, h, qsl],
                                                 rhs=k_hb[:, ksl], start=True, stop=True)
                                if kc == nfull:
                                    nc.vector.tensor_add(
                                        mx[:, r * P:(r + 1) * P],
                                        mx[:, r * P:(r + 1) * P], triu_sb[:])
                                nc.vector.tensor_reduce(
                                    mparts[:, kc:kc + 1], mx[:, :w], axis=AX, op=ALU.max)
                            negm = smpool.tile([P, 1], F32, tag="negm")
                            nc.vector.tensor_reduce(
                                negm[:], mparts[:, :nfull + 1], axis=AX, op=ALU.max,
                                negate=True)
                            bias_t = smpool.tile([P, 1], F32, tag="bias")
                            nc.vector.tensor_scalar_mul(bias_t[:], negm[:], SCALING)
                            # --- f32r scores; exp immediately (fp16 weights) ---
                            pu = pupool.tile([P, S], FP16, tag="pu")
                            zparts = smpool.tile([P, 8], F32, tag="zp")
                            for kc in range(nfull + 1):
                                w = 512 if kc < nfull else (r + 1) * P
                                ksl = slice(kc * 512, kc * 512 + w)
                                ps = scp.tile([P, 512], F32)
                                nc.tensor.matmul(
                                    ps[:, :w], lhsT=q_hi[:, h, qsl], rhs=k_hi[:, ksl],
                                    start=True, stop=True)
                                if kc == nfull:
                                    nc.vector.tensor_add(
                                        ps[:, r * P:(r + 1) * P],
                                        ps[:, r * P:(r + 1) * P], triu_sb[:])
                                nc.scalar.activation(
                                    pu[:, kc * 512:kc * 512 + w], ps[:, :w], AF.Exp,
                                    scale=SCALING, bias=bias_t[:],
                                    accum_out=zparts[:, kc:kc + 1])
                            zsum = smpool.tile([P, 1], F32, tag="zs")
                            nc.vector.tensor_reduce(
                                zsum[:], zparts[:, :nfull + 1], axis=AX, op=ALU.add)
                            rz = smpool.tile([P, 1], F32, tag="rz")
                            nc.vector.reciprocal(rz[:], zsum[:])
                            dmat = ddpool.tile([P, P], FP16, tag="dm")
                            nc.vector.tensor_scalar_mul(
                                dmat[:], identf_sb[:], rz[:])
                            pu_l[(h, qb)] = pu
                            d_l[(h, qb)] = dmat
                            # previous q-chunk's Wo partials fill the gaps
                            emit_wo(2)
                    for h in range(HPC):
                        # --- PuT (normalized, fp16) + AV, pipelined by one
                        # k-block so the PE never waits on the puts copy ---
                        ot_ps = otp.tile([P, 512], F32)
                        puts_l = {}
                        for kb in range(kmax + 1):
                            putp = shp.tile([P, 512], F32, tag="shared", name="putp")
                            i0 = max(kb - 4 * qc, 0)
                            for j in range(i0, 4):
                                qb = 4 * qc + j
                                nc.tensor.matmul(
                                    putp[:, j * P:(j + 1) * P],
                                    lhsT=pu_l[(h, qb)][:, kb * P:(kb + 1) * P],
                                    rhs=d_l[(h, qb)][:],
                                    start=True, stop=True)
                            # q-positions before i0*P don't attend to block kb
                            puts = ptspool.tile([P, 512], FP16)
                            nc.any.tensor_copy(puts[:, i0 * P:], putp[:, i0 * P:])
                            puts_l[kb] = (puts, i0)
                            if kb > 0:
                                pv, pi0 = puts_l.pop(kb - 1)
                                nc.tensor.matmul(
                                    ot_ps[:, pi0 * P:], lhsT=v_r[:, kb - 1, :],
                                    rhs=pv[:, pi0 * P:],
                                    start=(kb - 1 == 0), stop=False)
                        pv, pi0 = puts_l.pop(kmax)
                        nc.tensor.matmul(
                            ot_ps[:, pi0 * P:], lhsT=v_r[:, kmax, :],
                            rhs=pv[:, pi0 * P:],
                            start=(kmax == 0), stop=True)
                        csl = slice(qc * 512, (qc + 1) * 512)
                        nc.vector.tensor_mul(gated_r[:, h, csl], ot_ps[:],
                                             sig_r[:, h, csl])
                    # queue this q-chunk's Wo partials; they run inside the
                    # next q-chunk's softmax rounds (flush at the end)
                    for sb in range(4 * qc, 4 * qc + 4):
                        for dc in range(NSC):
                            wo_jobs.append((sb, dc))
                emit_wo(len(wo_jobs))

    nc.compile()
    return nc


def _host_inputs(hidden_states, cos, sin, Wq, Wk, Wv, Wo, q_norm_w, k_norm_w):
    hs = np.asarray(hidden_states, dtype=np.float32).reshape(S, DM)
    # pre-round the Q/K-critical operands so engine-side f32r rounding mode
    # cannot bias them; linear-path operands go in raw
    hsT = _round_fp32r(np.ascontiguousarray(hs.T))
    cosT = np.ascontiguousarray(np.asarray(cos, np.float32).T)
    sinT = np.ascontiguousarray(np.asarray(sin, np.float32).T)
    sinTs = sinT.copy()
    sinTs[:D // 2] = -sinTs[:D // 2]     # sign of rotate-half folded into sin
    ident = np.eye(P, dtype=np.float32)
    onec = np.ones((P, 1), np.float32)
    oner = np.ones((1, P), np.float32)
    triu = np.triu(np.full((P, P), -1e9, np.float32), 1)
    mask5 = np.zeros((P, 5 * 512), np.float32)
    for r in range(4):
        mask5[:, r * 512 + r * P:r * 512 + (r + 1) * P] = triu
    Wq = np.asarray(Wq, np.float32)
    Wk = np.asarray(Wk, np.float32)
    Wv = np.asarray(Wv, np.float32)
    Wo = np.asarray(Wo, np.float32)
    maps = []
    for c in range(NCORES):
        heads = [2 * c, 2 * c + 1]
        g = c // 2
        wq_c = np.concatenate([Wq[:, h * 2 * D:h * 2 * D + D] for h in heads], axis=1)
        wg_c = np.concatenate([Wq[:, h * 2 * D + D:(h + 1) * 2 * D] for h in heads], axis=1)
        maps.append({
            "hsT": hsT,
            "wq": _round_fp32r(wq_c),
            "wk": _round_fp32r(Wk[:, g * D:(g + 1) * D]),
            "wg": np.ascontiguousarray(wg_c),
            "wv": np.ascontiguousarray(Wv[:, g * D:(g + 1) * D]),
            "wo": np.ascontiguousarray(Wo[c * 2 * D:(c + 1) * 2 * D, :]).astype(np.float16),
            "cosT": cosT, "sinTs": sinTs,
            "qw": np.asarray(q_norm_w, np.float32).reshape(P, 1),
            "kw": np.asarray(k_norm_w, np.float32).reshape(P, 1),
            "ident": ident, "identf": ident, "onec": onec, "oner": oner,
            "onerr": oner, "triu": triu, "mask5": mask5,
        })
    return maps


def kernel(**inputs):
    from concourse.bass_utils import run_bass_kernel_spmd

    if "nc" not in _cache:
        _cache["nc"] = _build_nc()
    nc = _cache["nc"]
    maps = _host_inputs(
        inputs["hidden_states"], inputs["cos"], inputs["sin"],
        inputs["Wq"], inputs["Wk"], inputs["Wv"], inputs["Wo"],
        inputs["q_norm_w"], inputs["k_norm_w"])
    res = run_bass_kernel_spmd(nc, maps, list(range(NCORES)))
    total = np.zeros((S, DM), np.float64)
    for r in res.results:
        total += r["out"].astype(np.float64)
    return total.astype(np.float32).reshape(1, S, DM)
